# revision 15
# baseline (speedup 1.0000x reference)
"""Trainium2 Bass kernel for nn_BertSelfAttention (selective candidate scores).

kernel(**inputs) takes the FULL unsharded inputs (as from setup_inputs()) and
returns (context, selective_att_scores, selective_att_scores_r, probs),
matching the jax reference.

Sharding: data-parallel over batch B=8 across 8 NeuronCores (one example per
core, SPMD - one program, per-core input maps). No collectives.

Per-core algorithm (matmuls in fp32r = full-rate on the PE):
  phase 1:
    hT = hidden^T (PE transpose blocks)
    QTs[h] [66,S]: rows 0-63 = ((Wq/8)^T hidden^T + bq/8) head h, row 64 = ones,
                   row 65 = -log(softmax denom) (written in phase 2)
    KTa[h] [66,S]: rows 0-63 = (Wk^T hidden^T + bk) + 8*seg_rep^T, row 64 =
                   extra[k] = b_q_s.seg_rep[k] + mask[k], row 65 = ones
    V [S,D] natural (hidden Wv + bv, bias via rank-1 matmul channel)
  phase 2 per head:
    Q-side per q-tile: scores = QTs[0:65].T @ KTa[0:65]  (K=65 folds the
      rank-1 extra term); exp on ACT with accum_out row-sums; reciprocal;
      normalize on GpSimd; DMA probs out. -lse = ln(recip) per q.
    -lse cols -> PE transpose -> row 65 of QTs.
    T-side per k-tile: scoresT = KTa[0:66].T @ QTs[0:66]; channel 64 adds
      extra[k], channel 65 adds -lse[q], so exp(scoresT) is the exactly
      normalized probs^T. Context accumulates ctxT += V_h^T-form matmuls.
      Selective maxima via DVE tensor_mask_reduce over runtime [a_i, a_j)
      windows (att_ids arrive as f32 input values - fully data-driven SPMD).
    Segment means via tiny mask matmuls (masks pre-scaled by 1/count on host).
    ctxT -> PE transpose -> context tiles -> DMA.
"""

import os
import numpy as np
from contextlib import ExitStack
from dataclasses import dataclass

import concourse.bass as bass
import concourse.bacc as bacc
import concourse.bass_isa as bass_isa
import concourse.tile as tile
from concourse import mybir
from concourse.bass_utils import run_bass_kernel_spmd
from concourse.masks import make_identity

F32 = mybir.dt.float32
F32R = mybir.dt.float32r
AF = mybir.ActivationFunctionType
ALU = mybir.AluOpType

P = 128
NSEG = 6


@dataclass(frozen=True)
class Cfg:
    S: int = 1024
    D: int = 768
    H: int = 12

    @property
    def DH(self):
        return self.D // self.H

    @property
    def S_T(self):
        return self.S // P

    @property
    def D_T(self):
        return self.D // P

    @property
    def schunks(self):
        return [(o, min(512, self.S - o)) for o in range(0, self.S, 512)]

    @property
    def dchunks(self):
        return [(o, min(512, self.D - o)) for o in range(0, self.D, 512)]

    @property
    def windows(self):
        S = self.S
        return [(0, S // 4), (S // 8, S // 2), (S // 4, 3 * S // 4), (S // 2, S)]

    @property
    def rev_kt(self):
        # k-tiles covering [0, S//4) (all k < a0 live here)
        return (self.S // 4 + P - 1) // P


def _r(ap):
    return ap.bitcast(F32R)


def _patch_act_tables():
    """Route Exp and Ln to the one table set containing both, so the ACT
    engine loads its spline tables once instead of thrashing between the
    per-function default sets (~1.3us per reload, ~150 reloads otherwise)."""
    import concourse.hw_specs as hw_specs
    if getattr(hw_specs, "_act_tbl_patched", False):
        return
    orig = hw_specs.get_activation_tables
    def patched(arch):
        t = orig(arch)
        if "natural_log_exp_and_others" in t:
            for name, fns in t.items():
                if name != "natural_log_exp_and_others":
                    fns.discard(AF.Exp)
                    fns.discard(AF.Ln)
        return t
    hw_specs.get_activation_tables = patched
    hw_specs._act_tbl_patched = True
    import concourse.bacc as _bacc
    if getattr(_bacc, "get_activation_tables", None) is not None:
        _bacc.get_activation_tables = patched


def build_program(cfg: Cfg, n_cores: int):
    _patch_act_tables()
    S, D, H, DH = cfg.S, cfg.D, cfg.H, cfg.DH
    nc = bacc.Bacc("TRN2", target_bir_lowering=False, debug=False,
                   enable_asserts=True, num_devices=n_cores)

    io = {}
    def inp(name, shape):
        io[name] = nc.dram_tensor(name, shape, F32, kind="ExternalInput").ap()
    def outp(name, shape):
        io[name] = nc.dram_tensor(name, shape, F32, kind="ExternalOutput").ap()

    inp("hidden", [S, D])
    inp("wqs", [D, D]); inp("wk", [D, D]); inp("wv", [D, D])
    inp("bqs", [1, D]); inp("bk", [1, D])
    inp("seg8", [NSEG, D])
    inp("onehot", [NSEG, S])
    inp("extra2", [H, 2, S])
    inp("segmask", [S, 4])
    inp("attv", [1, 8])
    inp("ones", [1, S])
    inp("rngmaskw", [4, 512])
    outp("probs_out", [H, S, S])
    outp("ctx_out", [S, D])
    outp("sela_out", [3, H])
    outp("selb_out", [H, 3])

    with tile.TileContext(nc) as tc:
        with ExitStack() as ctx:
            _body(ctx, tc, nc, cfg, io)
    nc.compile()
    return nc


def _body(ctx, tc, nc, cfg: Cfg, io):
    S, D, H, DH = cfg.S, cfg.D, cfg.H, cfg.DH
    S_T, D_T = cfg.S_T, cfg.D_T

    const = ctx.enter_context(tc.tile_pool(name="const", bufs=1))
    heads = ctx.enter_context(tc.tile_pool(name="heads", bufs=1))
    ps_att = ctx.enter_context(tc.tile_pool(name="ps_att", bufs=2, space="PSUM"))
    ps_ctx = ctx.enter_context(tc.tile_pool(name="ps_ctx", bufs=1, space="PSUM"))
    ps_sm = ctx.enter_context(tc.tile_pool(name="ps_sm", bufs=1, space="PSUM"))
    ps_sel = ctx.enter_context(tc.tile_pool(name="ps_sel", bufs=1, space="PSUM"))

    ident = const.tile([P, P], F32)
    make_identity(nc, ident)


    bqs_c = const.tile([P, D_T], F32, tag="bqs")
    nc.sync.dma_start(bqs_c, io["bqs"].rearrange("o (t p) -> (o p) t", p=P))
    bk_c = const.tile([P, D_T], F32, tag="bk")
    nc.sync.dma_start(bk_c, io["bk"].rearrange("o (t p) -> (o p) t", p=P))
    attv = const.tile([P, 8], F32, tag="attv")
    nc.sync.dma_start(attv, io["attv"].partition_broadcast(P))
    maskw = const.tile([P, 4, 512], F32, tag="maskw")
    nc.sync.dma_start(maskw, io["rngmaskw"].partition_broadcast(P))
    segmask = const.tile([P, S_T, 4], F32, tag="segmask")
    nc.sync.dma_start(segmask, io["segmask"].rearrange("(t p) c -> p t c", p=P))
    oh_sb = const.tile([NSEG, S], F32R, tag="oh")
    nc.sync.dma_start(oh_sb, io["onehot"].bitcast(F32R))
    seg_sb = const.tile([NSEG, D], F32R, tag="seg")
    nc.sync.dma_start(seg_sb, io["seg8"].bitcast(F32R))

    QTs = [heads.tile([66, S], F32R, tag=f"qts{h}", name=f"qts{h}") for h in range(H)]
    KTa = [heads.tile([66, S], F32R, tag=f"kta{h}", name=f"kta{h}") for h in range(H)]
    V = heads.tile([P, S_T, D], F32R, tag="v")
    selA = heads.tile([3, H], F32, tag="selA")
    selB = heads.tile([1, H * 3], F32, tag="selB")

    for h in range(H):
        nc.sync.dma_start(QTs[h][64:65, :], io["ones"].bitcast(F32R))
        # rows 64-65 = [extra; ones] (base-64 aligned write)
        nc.sync.dma_start(KTa[h][64:66, :], io["extra2"][h].bitcast(F32R))

    # ---------------- phase 1 ----------------
    with tc.tile_pool(name="ph1T", bufs=1) as ph1T, \
         tc.tile_pool(name="ph1w", bufs=1) as ph1:
        # transpose hidden -> hT
        hT = None
        with tc.tile_pool(name="ph1h", bufs=2) as ph1h:
            hT_t = ph1T.tile([P, D_T, S], F32R, tag="hT")
            for st in range(S_T):
                h_t = ph1h.tile([P, D], F32, tag="h")
                nc.sync.dma_start(h_t, io["hidden"][st * P:(st + 1) * P, :])
                for dt in range(D_T):
                    pt = ps_sm.tile([P, P], F32, tag="sm")
                    nc.tensor.transpose(pt, h_t[:, dt * P:(dt + 1) * P], ident)
                    nc.vector.tensor_copy(hT_t[:, dt, st * P:(st + 1) * P], pt)
            hT = hT_t

        # Q / K projections into per-head transposed layouts
        for which in ("q", "k"):
            w_sb = ph1.tile([P, D_T, D], F32R, tag="w")
            nc.sync.dma_start(
                w_sb, io["wqs" if which == "q" else "wk"]
                .bitcast(F32R).rearrange("(t p) d -> p t d", p=P))
            bias_c = bqs_c if which == "q" else bk_c
            dst = QTs if which == "q" else KTa
            for dt in range(D_T):
                for (qo, qn) in cfg.schunks:
                    pt = ps_att.tile([P, S], F32, tag="att")
                    for ct in range(D_T):
                        nc.tensor.matmul(
                            pt[:, qo:qo + qn],
                            w_sb[:, ct, dt * P:(dt + 1) * P],
                            hT[:, ct, qo:qo + qn],
                            start=(ct == 0),
                            stop=(ct == D_T - 1 and which == "q"))
                    if which == "k":
                        nc.tensor.matmul(
                            pt[:, qo:qo + qn],
                            seg_sb[:, dt * P:(dt + 1) * P],
                            oh_sb[:, qo:qo + qn],
                            start=False, stop=True)
                    for half in range(2):
                        hh = 2 * dt + half
                        if hh >= H:
                            continue
                        nc.scalar.activation(
                            dst[hh][0:64, qo:qo + qn],
                            pt[half * 64:half * 64 + 64, qo:qo + qn],
                            AF.Identity,
                            bias=bias_c[half * 64:half * 64 + 64, dt:dt + 1])

        # V projection (natural layout) with rank-1 bias channel
        w_sb = ph1.tile([P, D_T, D], F32R, tag="w")
        nc.sync.dma_start(w_sb, io["wv"].bitcast(F32R).rearrange("(t p) d -> p t d", p=P))
        for st in range(S_T):
            for (do, dn) in cfg.dchunks:
                pt = ps_att.tile([P, S], F32, tag="att")
                for ct in range(D_T):
                    nc.tensor.matmul(
                        pt[:, 0:dn],
                        hT[:, ct, st * P:(st + 1) * P],
                        w_sb[:, ct, do:do + dn],
                        start=(ct == 0), stop=(ct == D_T - 1))
                nc.vector.tensor_copy(V[:, st, do:do + dn], pt[:, 0:dn])

    # ---------------- phase 2 ----------------
    work = ctx.enter_context(tc.tile_pool(name="work", bufs=2))
    small = ctx.enter_context(tc.tile_pool(name="small", bufs=2))
    ctx_sb = work.tile([P, S_T, D], F32, tag="ctx", bufs=1)
    a0_end = cfg.windows[0][1]
    for h in range(H):
        # ---- Q-side ----
        rc = small.tile([P, S_T], F32, tag="rc")
        nl2 = small.tile([P, S_T, 2], F32, tag="nl2")
        nc.vector.memset(nl2[:, :, 0:1], 1.0)
        for qt in range(S_T):
            psS = ps_att.tile([P, S], F32, tag="att")
            for (ko, kn) in cfg.schunks:
                nc.tensor.matmul(
                    psS[:, ko:ko + kn],
                    QTs[h][0:65, qt * P:(qt + 1) * P],
                    KTa[h][0:65, ko:ko + kn],
                    start=True, stop=True)
            eq = work.tile([P, S], F32, tag="eq")
            rsum = small.tile([P, 1], F32, tag="rsum")
            nc.scalar.activation(eq, psS, AF.Exp, accum_out=rsum)
            nc.vector.reciprocal(rc[:, qt:qt + 1], rsum)
            nc.scalar.activation(nl2[:, qt, 1:2], rc[:, qt:qt + 1], AF.Ln)
            prob = work.tile([P, S], F32, tag="prob")
            nc.vector.tensor_scalar_mul(prob, eq, rc[:, qt:qt + 1])
            nc.sync.dma_start(io["probs_out"][h, qt * P:(qt + 1) * P, :], prob)

        # [ones; -lse] pairs -> rows 64-65 of QTs[h] (base-64 aligned)
        for qt in range(S_T):
            p2 = ps_sm.tile([P, P], F32, tag="sm")
            nc.tensor.transpose(p2[0:2, :], nl2[:, qt, 0:2], ident)
            nc.vector.tensor_copy(
                QTs[h][64:66, qt * P:(qt + 1) * P], p2[0:2, :])

        # ---- T-side ----
        ctxT = ps_ctx.tile([64, S], F32, tag="ctxT")
        mq = small.tile([P, S_T], F32, tag="mq")
        mr = small.tile([P, 3 * cfg.rev_kt], F32, tag="mr")
        for kt in range(S_T):
            psT = ps_att.tile([P, S], F32, tag="att")
            for (qo, qn) in cfg.schunks:
                nc.tensor.matmul(
                    psT[:, qo:qo + qn],
                    KTa[h][0:66, kt * P:(kt + 1) * P],
                    QTs[h][0:66, qo:qo + qn],
                    start=True, stop=True)
            expT = work.tile([P, S], F32R, tag="expT")
            nc.scalar.activation(expT, psT, AF.Exp)
            for (qo, qn) in cfg.schunks:
                nc.tensor.matmul(
                    ctxT[:, qo:qo + qn],
                    V[:, kt, h * DH:(h + 1) * DH],
                    expT[:, qo:qo + qn],
                    start=(kt == 0), stop=(kt == S_T - 1))
            # forward maxima: max over q in [0, a0) -> mq[:, kt]
            if os.environ.get("DIAG_NO_MASKREDUCE") == "1":
                nc.vector.tensor_reduce(
                    mq[:, kt:kt + 1], expT[:, 0:a0_end].bitcast(F32),
                    axis=mybir.AxisListType.X, op=ALU.max)
                if kt < cfg.rev_kt:
                    for c in range(3):
                        wlo, whi = cfg.windows[c + 1]
                        nc.vector.tensor_reduce(
                            mr[:, kt * 3 + c:kt * 3 + c + 1],
                            expT[:, wlo:whi].bitcast(F32),
                            axis=mybir.AxisListType.X, op=ALU.max)
            else:
                # probs >= 0, so zeroing out-of-range q is exact for max
                mscr = work.tile([P, 512], F32, tag="mscr")
                nc.vector.tensor_mul(
                    mscr[:, 0:a0_end], expT[:, 0:a0_end].bitcast(F32),
                    maskw[:, 0, 0:a0_end])
                nc.vector.tensor_reduce(
                    mq[:, kt:kt + 1], mscr[:, 0:a0_end],
                    axis=mybir.AxisListType.X, op=ALU.max)
                if kt < cfg.rev_kt:
                    for c in range(3):
                        wlo, whi = cfg.windows[c + 1]
                        mscr2 = work.tile([P, 512], F32, tag="mscr")
                        nc.vector.tensor_mul(
                            mscr2[:, 0:whi - wlo], expT[:, wlo:whi].bitcast(F32),
                            maskw[:, c + 1, 0:whi - wlo])
                        nc.vector.tensor_reduce(
                            mr[:, kt * 3 + c:kt * 3 + c + 1],
                            mscr2[:, 0:whi - wlo],
                            axis=mybir.AxisListType.X, op=ALU.max)

        # ---- selective segment means (tiny f32 matmuls) ----
        psA = ps_sel.tile([4, 4], F32, tag="sel")
        for kt in range(S_T):
            nc.tensor.matmul(psA[0:3, 0:1], segmask[:, kt, 0:3],
                             mq[:, kt:kt + 1],
                             start=(kt == 0), stop=(kt == S_T - 1))
        psB = ps_sel.tile([4, 4], F32, tag="sel")
        for kt in range(cfg.rev_kt):
            nc.tensor.matmul(psB[0:1, 0:3], segmask[:, kt, 3:4],
                             mr[:, kt * 3:kt * 3 + 3],
                             start=(kt == 0), stop=(kt == cfg.rev_kt - 1))
        nc.vector.tensor_copy(selA[0:3, h:h + 1], psA[0:3, 0:1])
        nc.vector.tensor_copy(selB[0:1, h * 3:h * 3 + 3], psB[0:1, 0:3])

        # ---- context: transpose ctxT -> [S, DH] tiles ----
        ctxT_sb = work.tile([64, S], F32, tag="ctxTsb")
        nc.vector.tensor_copy(ctxT_sb, ctxT)
        for qt in range(S_T):
            ptr = ps_sm.tile([P, P], F32, tag="sm")
            nc.tensor.transpose(
                ptr[:, 0:64], ctxT_sb[:, qt * P:(qt + 1) * P], ident[0:64, 0:64])
            nc.vector.tensor_copy(
                ctx_sb[:, qt, h * DH:(h + 1) * DH], ptr[:, 0:64])

    for qt in range(S_T):
        nc.sync.dma_start(io["ctx_out"][qt * P:(qt + 1) * P, :], ctx_sb[:, qt, :])
    nc.sync.dma_start(io["sela_out"], selA)
    nc.sync.dma_start(io["selb_out"], selB[0:1, :])


# ---------------------------------------------------------------------------
# host side
# ---------------------------------------------------------------------------

def host_inputs(cfg: Cfg, b, hidden_states, attention_mask, seg_ids, att_ids,
                Wq, bq, Wk, bk, Wv, bv, seg_table, b_q_s):
    """Build the per-core input map for example b."""
    S, D, H, DH = cfg.S, cfg.D, cfg.H, cfg.DH
    inv8 = np.float32(1.0 / np.sqrt(DH))
    f = np.float32

    mask1d = np.asarray(attention_mask[b, 0, 0, :], dtype=f)       # [S]
    sid = np.asarray(seg_ids[b])                                    # [S]
    a = [int(x) for x in np.asarray(att_ids[b])]                    # 4

    tab = np.asarray(seg_table, dtype=f).reshape(NSEG, H, DH)
    bqs_flat = np.asarray(b_q_s, dtype=f).reshape(H, DH)
    tab_b = np.einsum("shd,hd->sh", tab, bqs_flat).astype(f)        # [6, H]
    bseg = tab_b[sid]                                               # [S, H]
    extra = bseg.T + mask1d[None, :]                                # [H, S]
    extra2 = np.stack([extra, np.ones_like(extra)], axis=1)         # [H, 2, S]

    onehot = np.zeros((NSEG, S), dtype=f)
    onehot[sid, np.arange(S)] = 1.0

    segmask = np.zeros((S, 4), dtype=f)
    iota = np.arange(S)
    for c in range(3):
        m = (iota >= a[c]) & (iota < a[c + 1])
        segmask[m, c] = 1.0 / f(a[c + 1] - a[c])
    segmask[iota < a[0], 3] = 1.0 / f(a[0])

    W = cfg.windows
    rngmaskw = np.zeros((4, 512), dtype=f)
    w0 = W[0][1]
    rngmaskw[0, :w0] = (np.arange(w0) < a[0])
    for c in range(3):
        wlo, whi = W[c + 1]
        ww = whi - wlo
        rngmaskw[c + 1, :ww] = ((np.arange(ww) + wlo >= a[c]) &
                                (np.arange(ww) + wlo < a[c + 1]))
    attv = np.array([[a[0],
                      a[0] - W[1][0], a[1] - W[1][0],
                      a[1] - W[2][0], a[2] - W[2][0],
                      a[2] - W[3][0], a[3] - W[3][0], 0.0]], dtype=f)

    return dict(
        hidden=np.ascontiguousarray(hidden_states[b], dtype=f),
        wqs=np.ascontiguousarray(np.asarray(Wq, dtype=f) * inv8),
        wk=np.ascontiguousarray(Wk, dtype=f),
        wv=np.ascontiguousarray(Wv, dtype=f),
        bqs=(np.asarray(bq, dtype=f) * inv8).reshape(1, D),
        bk=np.asarray(bk, dtype=f).reshape(1, D),
        # KTa rows = K + 8*seg_rep so that (Q/8).KTa = Q.K/8 + Q.seg_rep
        seg8=np.ascontiguousarray(np.asarray(seg_table, dtype=f) * 8.0),
        onehot=onehot,
        ones=np.ones((1, S), dtype=f),
        rngmaskw=rngmaskw,
        extra2=np.ascontiguousarray(extra2),
        segmask=segmask,
        attv=attv,
    )


_PROG_CACHE = {}
_BV = None
TRACE = False
LAST_RESULT = None


def _get_program(cfg: Cfg, n_cores: int):
    key = (cfg, n_cores)
    if key not in _PROG_CACHE:
        _PROG_CACHE[key] = build_program(cfg, n_cores)
    return _PROG_CACHE[key]


def kernel(hidden_states, attention_mask, seg_ids, att_ids,
           Wq, bq, Wk, bk, Wv, bv, seg_table, b_q_s):
    global _BV
    _BV = np.asarray(bv, dtype=np.float32)
    cfg = Cfg(S=np.asarray(hidden_states).shape[1],
              D=np.asarray(hidden_states).shape[2],
              H=12)
    B = np.asarray(hidden_states).shape[0]
    n_cores = 8
    nc = _get_program(cfg, n_cores)

    in_maps = [host_inputs(cfg, b, hidden_states, attention_mask, seg_ids,
                           att_ids, Wq, bq, Wk, bk, Wv, bv, seg_table, b_q_s)
               for b in range(B)]
    global LAST_RESULT
    kw = dict(trace=True, trace_cores=[0]) if TRACE else {}
    res = run_bass_kernel_spmd(nc, in_maps, core_ids=list(range(n_cores)), **kw)
    LAST_RESULT = res
    return assemble_outputs(cfg, B, [res.results[b] for b in range(B)])


def assemble_outputs(cfg: Cfg, B, outs):
    S, D, H = cfg.S, cfg.D, cfg.H
    context = np.stack([o["ctx_out"] for o in outs])                 # [B,S,D]
    context = context + _BV[None, None, :]
    probs = np.stack([o["probs_out"] for o in outs])                 # [B,H,S,S]
    sel = np.stack([o["sela_out"].T for o in outs])                  # [B,H,3]
    sel_r = np.stack([o["selb_out"] for o in outs])                  # [B,H,3]
    return context, sel, sel_r, probs
